# revision 48
# baseline (speedup 1.0000x reference)
"""NeRF volumetric alpha-compositing kernel for Trainium2 (Bass/Tile).

Full inputs:  rgbo [131072, 128, 4] f32, depth [131072, 128] f32.
Full output:  [131072, 3] f32.

Sharding: data-parallel over rays, 8 cores x 16384 rays. Host-side prep
per shard: rgb channels split out channel-major and cast to bf16 (the
on-chip tanh emits bf16 anyway; HBM traffic drops 42->30 MB/core),
opacity split out in f32 for the exact cumsum.

Per-core algorithm, ray-per-partition layout (BLOCK=128 rays on partitions,
T=8 rays per partition per superblock, S=128 samples each on the free dim):

  delta[s] = depth[s+1]-depth[s]; m[s] = opacity[s]*delta[s]; m[S-1] = 8.0
  cs       = inclusive_cumsum(m) over the whole T*S free extent (ONE scan;
             the soft sentinel 8.0 keeps the cross-ray accumulation finite:
             cs <= 7*(4+8) = 84 < ln(f32max))
  te[j]    = exp(-cs[j-1]), te[0] = 1        (ACT Exp, scale=-1)
  b[t]     = exp(+cs[t*S-1])                 (per-ray un-normalizer)
  w~[j]    = te[j]-te[j+1]                   (bf16; at each ray's last
             sample this equals te*(1-e^-8) ~ te, the FAR-sentinel weight)
  g[c]     = tanh(0.5*rgb_c)                 (bf16; sigmoid(x) =
             0.5+0.5*tanh(x/2) keeps ACT on one table set with Exp ->
             zero ACT_TABLE_LOAD churn)
  S_c[t]   = sum_s w~[t,s]*g[c][t,s]         (dense bf16 muls at DVE 2x
             mode + 2 bf16 fold levels + one segmented tensor_reduce)
  out[t,c] = (S_c[t]*b[t])*0.5 + 0.5

The per-ray factor b cancels the cross-ray accumulation of the single long
scan; sum_s w~ telescopes to 1/b exactly, so the sigmoid "+0.5" term
reduces to the constant 0.5 (folded into the final tensor_scalar).

Engine split (measured-driven): ALL bulk elementwise work on DVE - GPSIMD
shares an SBUF port with the DVE and offloading delta/m there taxed every
concurrent DVE op more than it saved. GPSIMD keeps only the tiny finals
and memsets; Exp+Tanh on ACT; input DMA on sync, output DMA on scalar
(second HWDGE ring). Emission is software-pipelined per round as
HEAD(n+1) [DMA, delta/m, scan], TAIL(n) [exp, w~, muls, folds, reduce],
ACT(n+1) [tanh, b-exp(n)], FINISH(n) [finals, out DMA] so the in-order
engine queues always hold independent work between dependent pairs.
"""

from contextlib import ExitStack

import numpy as np

import concourse.bass as bass
import concourse.tile as tile
from concourse import bacc, mybir
from concourse.bass_utils import run_bass_kernel_spmd

N_RAYS = 131072
S = 128
N_CORES = 8
NC_RAYS = N_RAYS // N_CORES  # 16384 rays per core
BLOCK = 128                  # rays per partition-block
F32 = mybir.dt.float32
BF16 = mybir.dt.bfloat16


def build_nerf_bass(
    n_rays: int = NC_RAYS,
    t_blocks: int = 8,
    bufs: int = 2,
    mid_bufs: int = 0,
    gpsimd_dm="none",
    gpsimd_finals: bool = True,
    fold: int = 2,
    out_dma: str = "scalar",
    g_psum: bool = False,
    cs_psum: bool = False,
    wg_psum: bool = False,
    scan_c1: bool = True,
    stage3: bool = True,
) -> bass.Bass:
    T = t_blocks
    SUPER = BLOCK * T
    assert n_rays % SUPER == 0
    n_super = n_rays // SUPER
    N = S * T  # free extent per partition
    # Soft sentinel only fits in f32's exp range for T=8 (cs <= 7*(4+8));
    # larger T uses sentinel 0 plus an explicit last-sample fix-up copy.
    SENT = 8.0 if T <= 8 else 0.0

    nc = bacc.Bacc("TRN2", target_bir_lowering=False, debug=False)
    # Host-side prep: rgb channels arrive channel-major and pre-cast to
    # bf16 (tanh output is bf16 anyway; cuts HBM traffic 42->30 MB/core),
    # opacity stays f32 (feeds the f32 cumsum).
    rgb_h = nc.declare_dram_parameter("rgb", [n_rays, 3, S], BF16, isOutput=False)
    opac_h = nc.declare_dram_parameter("opac", [n_rays, S], BF16, isOutput=False)
    depth_h = nc.declare_dram_parameter("depth", [n_rays, S], F32, isOutput=False)
    out_h = nc.declare_dram_parameter("out", [n_rays, 3], F32, isOutput=True)

    rgb_ap = rgb_h.ap()
    opac_ap = opac_h.ap()
    depth_ap = depth_h.ap()
    out_ap = out_h.ap()

    with ExitStack() as ctx:
        tc = ctx.enter_context(tile.TileContext(nc))
        p_in = ctx.enter_context(tc.tile_pool(name="inp", bufs=bufs))
        p_mid = ctx.enter_context(tc.tile_pool(name="mid", bufs=mid_bufs or bufs))
        p_out = ctx.enter_context(tc.tile_pool(name="outp", bufs=bufs))
        p_ps = (
            ctx.enter_context(tc.tile_pool(name="ps", bufs=bufs, space="PSUM"))
            if (g_psum or cs_psum or wg_psum)
            else None
        )

        # gpsimd_dm: "both" | "delta" | "none" - which of delta/m run on
        # GPSIMD. GPSIMD shares an SBUF port with VectorE, so offloading
        # there taxes every concurrent DVE op; "none" measured fastest.
        if gpsimd_dm is True:
            gpsimd_dm_mode = "both"
        elif gpsimd_dm is False:
            gpsimd_dm_mode = "none"
        else:
            gpsimd_dm_mode = gpsimd_dm
        eng_delta = nc.gpsimd if gpsimd_dm_mode in ("both", "delta") else nc.vector
        eng_m = nc.gpsimd if gpsimd_dm_mode == "both" else nc.vector
        eng_fin = nc.gpsimd if gpsimd_finals else nc.vector
        eng_odma = {"scalar": nc.scalar, "sync": nc.sync, "gpsimd": nc.gpsimd}[
            out_dma
        ]

        def emit_head(r0):
            """DMA in, tanh, delta/m, scan. Only depends on this sb's DMA."""
            rgb_t = p_in.tile([BLOCK, 3 * N], BF16, tag="rgb")
            o_t = p_in.tile([BLOCK, N], BF16, tag="opac")
            depth_t = p_in.tile([BLOCK, N], F32, tag="depth")
            nc.sync.dma_start(
                out=rgb_t,
                in_=rgb_ap[r0 : r0 + SUPER].rearrange(
                    "(p t) c s -> p (t c s)", p=BLOCK
                ),
            )
            nc.sync.dma_start(
                out=o_t,
                in_=opac_ap[r0 : r0 + SUPER].rearrange(
                    "(p t) s -> p (t s)", p=BLOCK
                ),
            )
            nc.sync.dma_start(
                out=depth_t,
                in_=depth_ap[r0 : r0 + SUPER].rearrange(
                    "(p t) s -> p (t s)", p=BLOCK
                ),
            )

            depth3 = depth_t.rearrange("p (t s) -> p t s", t=T)

            # te/b allocated here and preset on GPSIMD so the exp in TAIL
            # never waits on a VectorE memset.
            te_t = p_mid.tile([BLOCK, N + 4], F32, tag="te")
            b_t = p_mid.tile([BLOCK, T], F32, tag="b")
            nc.gpsimd.memset(te_t[:, 0:1], 1.0)
            nc.gpsimd.memset(b_t[:, 0:1], 1.0)

            # delta / m computed FLAT over the whole extent (cross-ray
            # garbage at each ray's last sample is overwritten by the
            # sentinel memset). delta is cast to bf16 after the exact f32
            # sub, opacity arrives bf16 from the host, so the m multiply
            # is a dense bf16 TT at the DVE 2x mode.
            delta_t = p_mid.tile([BLOCK, N], BF16, tag="delta")
            eng_delta.tensor_sub(
                delta_t[:, 0 : N - 1], depth_t[:, 1:N], depth_t[:, 0 : N - 1]
            )
            m_t = p_mid.tile([BLOCK, N], BF16, tag="m")
            m3 = m_t.rearrange("p (t s) -> p t s", t=T)
            eng_m.tensor_mul(
                m_t[:, 0 : N - 1],
                delta_t[:, 0 : N - 1],
                o_t[:, 0 : N - 1],
            )
            # soft sentinel: te drops by e^-8 at each ray boundary, so the
            # plain w~ difference already yields the last sample's weight
            # (te[last], to 3e-4 relative) and no fix-up op is needed. The
            # per-ray un-normalizer b = exp(+cs_boundary) stays finite:
            # cs <= 7*(4+8) = 84 < ln(f32max), and bf16 wg values stay
            # above bf16's min normal (e^-84 = 3e-37 > 1.2e-38).
            eng_m.memset(m3[:, :, S - 1], SENT)

            # one inclusive scan over the whole T*S extent
            cs_t = (p_ps if cs_psum else p_mid).tile([BLOCK, N], F32, tag="cs")
            scan_d1 = (
                nc.const_aps.tensor(0.0, (BLOCK, N)) if scan_c1 else m_t[:]
            )
            nc.vector.tensor_tensor_scan(
                cs_t[:],
                m_t[:],
                scan_d1,
                0.0,
                mybir.AluOpType.add,
                mybir.AluOpType.bypass,
            )
            return r0, rgb_t, cs_t, te_t, b_t

        def emit_act(state, prev_b=None):
            """tanh for this sb - emitted AFTER the previous sb's TAIL so
            ACT's in-order queue serves exp(n) before tanh(n+1). The
            previous sb's b-exp rides behind this tanh (b is only needed
            by the finals, much later than w~ needs exp)."""
            r0, rgb_t, cs_t, te_t, b_t = state
            # g = tanh(rgb/2) in one flat dense->dense ACT op (1 elem/cyc).
            # Layout (t c s): per-channel views are dense 128-runs -> the
            # bf16 wg muls get the 2x DVE mode.
            g_t = (p_ps if g_psum else p_mid).tile([BLOCK, 3 * N], BF16, tag="g")
            nc.scalar.activation(
                g_t[:],
                rgb_t[:],
                mybir.ActivationFunctionType.Tanh,
                scale=0.5,
            )
            if prev_b is not None:
                prev_cs3, prev_b_t = prev_b
                nc.scalar.activation(
                    prev_b_t[:, 1:T],
                    prev_cs3[:, 0 : T - 1, S - 1],
                    mybir.ActivationFunctionType.Exp,
                )
            return r0, g_t, cs_t, te_t, b_t

        def emit_tail(state, late_b=False):
            r0, g_t, cs_t, te_t, b_t = state
            # te[j] = exp(-cs[j-1]), te[0] = 1
            nc.scalar.activation(
                te_t[:, 1 : N + 1],
                cs_t[:],
                mybir.ActivationFunctionType.Exp,
                scale=-1.0,
            )

            # per-ray un-normalizer b[t] = exp(+cs[t*S-1]); the sigmoid
            # 0.5-affine is folded into the final tensor_scalar instead.
            cs3 = cs_t.rearrange("p (t s) -> p t s", t=T)
            if not late_b:
                nc.scalar.activation(
                    b_t[:, 1:T],
                    cs3[:, 0 : T - 1, S - 1],
                    mybir.ActivationFunctionType.Exp,
                )
            # w~[j] = te[j]-te[j+1]; the soft sentinel makes each ray's last
            # entry come out as te[last]*(1-e^-8) with no fix-up op. With
            # sentinel 0 (T>8) the last entry is 0 and is patched with an
            # explicit strided copy of te instead.
            w_t = p_mid.tile([BLOCK, N], BF16, tag="w")
            nc.vector.tensor_sub(w_t[:], te_t[:, 0:N], te_t[:, 1 : N + 1])
            if SENT == 0.0:
                te3v = te_t[:, 0:N].rearrange("p (t s) -> p t s", t=T)
                w3v = w_t.rearrange("p (t s) -> p t s", t=T)
                nc.vector.tensor_copy(w3v[:, :, S - 1], te3v[:, :, S - 1])

            # wg[c] = w~*g[c]  (all dense bf16 -> 2x DVE mode)
            gv = g_t.rearrange("p (t c s) -> p c t s", t=T, c=3)
            wg_t = (p_ps if wg_psum else p_mid).tile([BLOCK, 3 * N], BF16, tag="wg")
            wg3 = wg_t.rearrange("p (c t s) -> p c t s", c=3, t=T)
            wts = w_t.rearrange("p (t s) -> p t s", t=T)
            for c in range(3):
                nc.vector.tensor_mul(wg3[:, c], wts, gv[:, c])

            # segmented reduce over s -> S_c[t], layout [p, (c t)], with
            # `fold` levels of bf16 pairwise adds (2x mode) shrinking the
            # 1x-only tensor_reduce input first.
            s_t = p_mid.tile([BLOCK, 3 * T], F32, tag="s")
            red_in, seg = wg_t, S
            for lvl in range(fold):
                half = seg // 2
                f_t = p_mid.tile([BLOCK, 3 * T * half], BF16, tag=f"wgf{lvl}")
                nc.vector.tensor_add(
                    f_t.rearrange("p (n s) -> p n s", s=half),
                    red_in.rearrange("p (n s) -> p n s", s=seg)[:, :, 0:half],
                    red_in.rearrange("p (n s) -> p n s", s=seg)[:, :, half:seg],
                )
                red_in, seg = f_t, half
            nc.vector.tensor_reduce(
                s_t[:],
                red_in.rearrange("p (n s) -> p n s", s=seg),
                mybir.AxisListType.X,
                mybir.AluOpType.add,
            )

            return r0, s_t, b_t, cs3

        def emit_finish(fin_state):
            # out[t,c] = (S_c[t]*b[t])*0.5 + 0.5
            r0, s_t, b_t, cs3 = fin_state
            out_t = p_out.tile([BLOCK, 3 * T], F32, tag="out")
            out3 = out_t.rearrange("p (t c) -> p t c", c=3)
            s3 = s_t.rearrange("p (c t) -> p c t", c=3)
            for c in range(3):
                eng_fin.tensor_mul(out3[:, :, c], s3[:, c], b_t[:])
            out2_t = p_out.tile([BLOCK, 3 * T], F32, tag="out2")
            eng_fin.tensor_scalar(
                out2_t[:],
                out_t[:],
                0.5,
                0.5,
                mybir.AluOpType.mult,
                mybir.AluOpType.add,
            )

            eng_odma.dma_start(
                out=out_ap[r0 : r0 + SUPER].rearrange("(p t) c -> p (t c)", p=BLOCK),
                in_=out2_t[:],
            )

        # software pipeline: per round emit HEAD(n+1), TAIL(n), ACT(n+1)
        # (which also carries b-exp(n)), then FINISH(n). Every in-order
        # engine queue holds independent work between dependent pairs; in
        # particular ACT's queue goes [exp(n), tanh(n+1), b(n), ...] so
        # w~(n) is unblocked as early as possible and b(n) still lands
        # before the finals read it.
        if stage3:
            pending = emit_act(emit_head(0))
            fin = None
            for sb in range(1, n_super):
                nxt = emit_head(sb * SUPER)
                fin = emit_tail(pending, late_b=True)
                pending = emit_act(nxt, prev_b=(fin[3], fin[2]))
                emit_finish(fin)
            fin = emit_tail(pending, late_b=True)
            nc.scalar.activation(
                fin[2][:, 1:T],
                fin[3][:, 0 : T - 1, S - 1],
                mybir.ActivationFunctionType.Exp,
            )
            emit_finish(fin)
        else:
            pending = emit_act(emit_head(0))
            for sb in range(1, n_super):
                nxt = emit_act(emit_head(sb * SUPER))
                emit_finish(emit_tail(pending))
                pending = nxt
            emit_finish(emit_tail(pending))
    nc.compile()
    return nc


_NC_CACHE: dict = {}


def _get_nc(**kwargs):
    key = tuple(sorted(kwargs.items()))
    if key not in _NC_CACHE:
        _NC_CACHE[key] = build_nerf_bass(**kwargs)
    return _NC_CACHE[key]


def kernel(rgbo: np.ndarray, depth: np.ndarray, build_kwargs=None, **run_kwargs) -> np.ndarray:
    rgbo = np.ascontiguousarray(rgbo, dtype=np.float32)
    depth = np.ascontiguousarray(depth, dtype=np.float32)
    assert rgbo.shape == (N_RAYS, S, 4) and depth.shape == (N_RAYS, S)

    nc = _get_nc(**(build_kwargs or {}))
    # Host-side prep: channel-major rgb pre-cast to bf16 (the on-chip tanh
    # emits bf16 anyway), opacity split out in f32 for the exact cumsum.
    import ml_dtypes

    rgbo_cm = rgbo.transpose(0, 2, 1)  # [N, 4, S] view
    rgb_bf = np.ascontiguousarray(rgbo_cm[:, :3]).astype(ml_dtypes.bfloat16)
    opac = np.ascontiguousarray(rgbo_cm[:, 3]).astype(ml_dtypes.bfloat16)
    in_maps = []
    for i in range(N_CORES):
        sl = slice(i * NC_RAYS, (i + 1) * NC_RAYS)
        in_maps.append({"rgb": rgb_bf[sl], "opac": opac[sl], "depth": depth[sl]})
    res = run_bass_kernel_spmd(nc, in_maps, core_ids=list(range(N_CORES)), **run_kwargs)
    out = np.concatenate([r["out"] for r in res.results], axis=0)
    if run_kwargs:
        kernel.last_results = res  # stash for profiling harnesses
    return out


# revision 50
# speedup vs baseline: 1.1489x; 1.1489x over previous
"""NeRF volumetric alpha-compositing kernel for Trainium2 (Bass/Tile).

Full inputs:  rgbo [131072, 128, 4] f32, depth [131072, 128] f32.
Full output:  [131072, 3] f32.

Sharding: data-parallel over rays, 8 cores x 16384 rays. Host-side prep
per shard: rgb channels split out channel-major and cast to bf16 (the
on-chip tanh emits bf16 anyway; HBM traffic drops 42->30 MB/core),
opacity split out in f32 for the exact cumsum.

Per-core algorithm, ray-per-partition layout (BLOCK=128 rays on partitions,
T=8 rays per partition per superblock, S=128 samples each on the free dim):

  delta[s] = depth[s+1]-depth[s]; m[s] = opacity[s]*delta[s]; m[S-1] = 8.0
  cs       = inclusive_cumsum(m) over the whole T*S free extent (ONE scan;
             the soft sentinel 8.0 keeps the cross-ray accumulation finite:
             cs <= 7*(4+8) = 84 < ln(f32max))
  te[j]    = exp(-cs[j-1]), te[0] = 1        (ACT Exp, scale=-1)
  b[t]     = exp(+cs[t*S-1])                 (per-ray un-normalizer)
  w~[j]    = te[j]-te[j+1]                   (bf16; at each ray's last
             sample this equals te*(1-e^-8) ~ te, the FAR-sentinel weight)
  g[c]     = tanh(0.5*rgb_c)                 (bf16; sigmoid(x) =
             0.5+0.5*tanh(x/2) keeps ACT on one table set with Exp ->
             zero ACT_TABLE_LOAD churn)
  S_c[t]   = sum_s w~[t,s]*g[c][t,s]         (dense bf16 muls at DVE 2x
             mode + 2 bf16 fold levels + one segmented tensor_reduce)
  out[t,c] = (S_c[t]*b[t])*0.5 + 0.5

The per-ray factor b cancels the cross-ray accumulation of the single long
scan; sum_s w~ telescopes to 1/b exactly, so the sigmoid "+0.5" term
reduces to the constant 0.5 (folded into the final tensor_scalar).

Engine split (measured-driven): ALL bulk elementwise work on DVE - GPSIMD
shares an SBUF port with the DVE and offloading delta/m there taxed every
concurrent DVE op more than it saved. GPSIMD keeps only the tiny finals
and memsets; Exp+Tanh on ACT; input DMA on sync, output DMA on scalar
(second HWDGE ring). Emission is software-pipelined per round as
HEAD(n+1) [DMA, delta/m, scan], TAIL(n) [exp, w~, muls, folds, reduce],
ACT(n+1) [tanh, b-exp(n)], FINISH(n) [finals, out DMA] so the in-order
engine queues always hold independent work between dependent pairs.
"""

from contextlib import ExitStack

import numpy as np

import concourse.bass as bass
import concourse.tile as tile
from concourse import bacc, mybir
from concourse.bass_utils import run_bass_kernel_spmd

N_RAYS = 131072
S = 128
N_CORES = 8
NC_RAYS = N_RAYS // N_CORES  # 16384 rays per core
BLOCK = 128                  # rays per partition-block
F32 = mybir.dt.float32
BF16 = mybir.dt.bfloat16


def build_nerf_bass(
    n_rays: int = NC_RAYS,
    t_blocks: int = 8,
    bufs: int = 2,
    mid_bufs: int = 0,
    gpsimd_dm="none",
    gpsimd_finals: bool = True,
    fold: int = 2,
    out_dma: str = "scalar",
    g_psum: bool = False,
    cs_psum: bool = False,
    wg_psum: bool = False,
    scan_c1: bool = True,
    stage3: bool = True,
) -> bass.Bass:
    T = t_blocks
    SUPER = BLOCK * T
    assert n_rays % SUPER == 0
    n_super = n_rays // SUPER
    N = S * T  # free extent per partition
    # Soft sentinel only fits in f32's exp range for T=8 (cs <= 7*(4+8));
    # larger T uses sentinel 0 plus an explicit last-sample fix-up copy.
    SENT = 8.0 if T <= 8 else 0.0

    nc = bacc.Bacc("TRN2", target_bir_lowering=False, debug=False)
    # Host-side prep: rgb channels arrive channel-major and pre-cast to
    # bf16 (tanh output is bf16 anyway; cuts HBM traffic 42->30 MB/core),
    # opacity stays f32 (feeds the f32 cumsum).
    rgb_h = nc.declare_dram_parameter("rgb", [n_rays, 3, S], BF16, isOutput=False)
    opac_h = nc.declare_dram_parameter("opac", [n_rays, S], F32, isOutput=False)
    depth_h = nc.declare_dram_parameter("depth", [n_rays, S], F32, isOutput=False)
    out_h = nc.declare_dram_parameter("out", [n_rays, 3], F32, isOutput=True)

    rgb_ap = rgb_h.ap()
    opac_ap = opac_h.ap()
    depth_ap = depth_h.ap()
    out_ap = out_h.ap()

    with ExitStack() as ctx:
        tc = ctx.enter_context(tile.TileContext(nc))
        p_in = ctx.enter_context(tc.tile_pool(name="inp", bufs=bufs))
        p_mid = ctx.enter_context(tc.tile_pool(name="mid", bufs=mid_bufs or bufs))
        p_out = ctx.enter_context(tc.tile_pool(name="outp", bufs=bufs))
        p_ps = (
            ctx.enter_context(tc.tile_pool(name="ps", bufs=bufs, space="PSUM"))
            if (g_psum or cs_psum or wg_psum)
            else None
        )

        # gpsimd_dm: "both" | "delta" | "none" - which of delta/m run on
        # GPSIMD. GPSIMD shares an SBUF port with VectorE, so offloading
        # there taxes every concurrent DVE op; "none" measured fastest.
        if gpsimd_dm is True:
            gpsimd_dm_mode = "both"
        elif gpsimd_dm is False:
            gpsimd_dm_mode = "none"
        else:
            gpsimd_dm_mode = gpsimd_dm
        eng_delta = nc.gpsimd if gpsimd_dm_mode in ("both", "delta") else nc.vector
        eng_m = nc.gpsimd if gpsimd_dm_mode == "both" else nc.vector
        eng_fin = nc.gpsimd if gpsimd_finals else nc.vector
        eng_odma = {"scalar": nc.scalar, "sync": nc.sync, "gpsimd": nc.gpsimd}[
            out_dma
        ]

        def emit_head(r0):
            """DMA in, tanh, delta/m, scan. Only depends on this sb's DMA."""
            rgb_t = p_in.tile([BLOCK, 3 * N], BF16, tag="rgb")
            o_t = p_in.tile([BLOCK, N], F32, tag="opac")
            depth_t = p_in.tile([BLOCK, N], F32, tag="depth")
            nc.sync.dma_start(
                out=rgb_t,
                in_=rgb_ap[r0 : r0 + SUPER].rearrange(
                    "(p t) c s -> p (t c s)", p=BLOCK
                ),
            )
            nc.sync.dma_start(
                out=o_t,
                in_=opac_ap[r0 : r0 + SUPER].rearrange(
                    "(p t) s -> p (t s)", p=BLOCK
                ),
            )
            nc.sync.dma_start(
                out=depth_t,
                in_=depth_ap[r0 : r0 + SUPER].rearrange(
                    "(p t) s -> p (t s)", p=BLOCK
                ),
            )

            depth3 = depth_t.rearrange("p (t s) -> p t s", t=T)

            # te/b allocated here and preset on GPSIMD so the exp in TAIL
            # never waits on a VectorE memset.
            te_t = p_mid.tile([BLOCK, N + 4], F32, tag="te")
            b_t = p_mid.tile([BLOCK, T], F32, tag="b")
            nc.gpsimd.memset(te_t[:, 0:1], 1.0)
            nc.gpsimd.memset(b_t[:, 0:1], 1.0)

            # delta / m (segmented 3D views; ray-boundary positions are
            # never computed, the sentinel memset writes them directly)
            o3 = o_t.rearrange("p (t s) -> p t s", t=T)
            delta_t = p_mid.tile([BLOCK, N], F32, tag="delta")
            delta3 = delta_t.rearrange("p (t s) -> p t s", t=T)
            eng_delta.tensor_sub(
                delta3[:, :, 0 : S - 1], depth3[:, :, 1:S], depth3[:, :, 0 : S - 1]
            )
            m_t = p_mid.tile([BLOCK, N], F32, tag="m")
            m3 = m_t.rearrange("p (t s) -> p t s", t=T)
            eng_m.tensor_mul(
                m3[:, :, 0 : S - 1],
                delta3[:, :, 0 : S - 1],
                o3[:, :, 0 : S - 1],
            )
            # soft sentinel: te drops by e^-8 at each ray boundary, so the
            # plain w~ difference already yields the last sample's weight
            # (te[last], to 3e-4 relative) and no fix-up op is needed. The
            # per-ray un-normalizer b = exp(+cs_boundary) stays finite:
            # cs <= 7*(4+8) = 84 < ln(f32max), and bf16 wg values stay
            # above bf16's min normal (e^-84 = 3e-37 > 1.2e-38).
            eng_m.memset(m3[:, :, S - 1], SENT)

            # one inclusive scan over the whole T*S extent
            cs_t = (p_ps if cs_psum else p_mid).tile([BLOCK, N], F32, tag="cs")
            scan_d1 = (
                nc.const_aps.tensor(0.0, (BLOCK, N)) if scan_c1 else m_t[:]
            )
            nc.vector.tensor_tensor_scan(
                cs_t[:],
                m_t[:],
                scan_d1,
                0.0,
                mybir.AluOpType.add,
                mybir.AluOpType.bypass,
            )
            return r0, rgb_t, cs_t, te_t, b_t

        def emit_act(state, prev_b=None):
            """tanh for this sb - emitted AFTER the previous sb's TAIL so
            ACT's in-order queue serves exp(n) before tanh(n+1). The
            previous sb's b-exp rides behind this tanh (b is only needed
            by the finals, much later than w~ needs exp)."""
            r0, rgb_t, cs_t, te_t, b_t = state
            # g = tanh(rgb/2) in one flat dense->dense ACT op (1 elem/cyc).
            # Layout (t c s): per-channel views are dense 128-runs -> the
            # bf16 wg muls get the 2x DVE mode.
            g_t = (p_ps if g_psum else p_mid).tile([BLOCK, 3 * N], BF16, tag="g")
            nc.scalar.activation(
                g_t[:],
                rgb_t[:],
                mybir.ActivationFunctionType.Tanh,
                scale=0.5,
            )
            if prev_b is not None:
                prev_cs3, prev_b_t = prev_b
                nc.scalar.activation(
                    prev_b_t[:, 1:T],
                    prev_cs3[:, 0 : T - 1, S - 1],
                    mybir.ActivationFunctionType.Exp,
                )
            return r0, g_t, cs_t, te_t, b_t

        def emit_tail(state, late_b=False):
            r0, g_t, cs_t, te_t, b_t = state
            # te[j] = exp(-cs[j-1]), te[0] = 1
            nc.scalar.activation(
                te_t[:, 1 : N + 1],
                cs_t[:],
                mybir.ActivationFunctionType.Exp,
                scale=-1.0,
            )

            # per-ray un-normalizer b[t] = exp(+cs[t*S-1]); the sigmoid
            # 0.5-affine is folded into the final tensor_scalar instead.
            cs3 = cs_t.rearrange("p (t s) -> p t s", t=T)
            if not late_b:
                nc.scalar.activation(
                    b_t[:, 1:T],
                    cs3[:, 0 : T - 1, S - 1],
                    mybir.ActivationFunctionType.Exp,
                )
            # w~[j] = te[j]-te[j+1]; the soft sentinel makes each ray's last
            # entry come out as te[last]*(1-e^-8) with no fix-up op. With
            # sentinel 0 (T>8) the last entry is 0 and is patched with an
            # explicit strided copy of te instead.
            w_t = p_mid.tile([BLOCK, N], BF16, tag="w")
            nc.vector.tensor_sub(w_t[:], te_t[:, 0:N], te_t[:, 1 : N + 1])
            if SENT == 0.0:
                te3v = te_t[:, 0:N].rearrange("p (t s) -> p t s", t=T)
                w3v = w_t.rearrange("p (t s) -> p t s", t=T)
                nc.vector.tensor_copy(w3v[:, :, S - 1], te3v[:, :, S - 1])

            # wg[c] = w~*g[c]  (all dense bf16 -> 2x DVE mode)
            gv = g_t.rearrange("p (t c s) -> p c t s", t=T, c=3)
            wg_t = (p_ps if wg_psum else p_mid).tile([BLOCK, 3 * N], BF16, tag="wg")
            wg3 = wg_t.rearrange("p (c t s) -> p c t s", c=3, t=T)
            wts = w_t.rearrange("p (t s) -> p t s", t=T)
            for c in range(3):
                nc.vector.tensor_mul(wg3[:, c], wts, gv[:, c])

            # segmented reduce over s -> S_c[t], layout [p, (c t)], with
            # `fold` levels of bf16 pairwise adds (2x mode) shrinking the
            # 1x-only tensor_reduce input first.
            s_t = p_mid.tile([BLOCK, 3 * T], F32, tag="s")
            red_in, seg = wg_t, S
            for lvl in range(fold):
                half = seg // 2
                f_t = p_mid.tile([BLOCK, 3 * T * half], BF16, tag=f"wgf{lvl}")
                nc.vector.tensor_add(
                    f_t.rearrange("p (n s) -> p n s", s=half),
                    red_in.rearrange("p (n s) -> p n s", s=seg)[:, :, 0:half],
                    red_in.rearrange("p (n s) -> p n s", s=seg)[:, :, half:seg],
                )
                red_in, seg = f_t, half
            nc.vector.tensor_reduce(
                s_t[:],
                red_in.rearrange("p (n s) -> p n s", s=seg),
                mybir.AxisListType.X,
                mybir.AluOpType.add,
            )

            return r0, s_t, b_t, cs3

        def emit_finish(fin_state):
            # out[t,c] = (S_c[t]*b[t])*0.5 + 0.5
            r0, s_t, b_t, cs3 = fin_state
            out_t = p_out.tile([BLOCK, 3 * T], F32, tag="out")
            out3 = out_t.rearrange("p (t c) -> p t c", c=3)
            s3 = s_t.rearrange("p (c t) -> p c t", c=3)
            for c in range(3):
                eng_fin.tensor_mul(out3[:, :, c], s3[:, c], b_t[:])
            out2_t = p_out.tile([BLOCK, 3 * T], F32, tag="out2")
            eng_fin.tensor_scalar(
                out2_t[:],
                out_t[:],
                0.5,
                0.5,
                mybir.AluOpType.mult,
                mybir.AluOpType.add,
            )

            eng_odma.dma_start(
                out=out_ap[r0 : r0 + SUPER].rearrange("(p t) c -> p (t c)", p=BLOCK),
                in_=out2_t[:],
            )

        # software pipeline: per round emit HEAD(n+1), TAIL(n), ACT(n+1)
        # (which also carries b-exp(n)), then FINISH(n). Every in-order
        # engine queue holds independent work between dependent pairs; in
        # particular ACT's queue goes [exp(n), tanh(n+1), b(n), ...] so
        # w~(n) is unblocked as early as possible and b(n) still lands
        # before the finals read it.
        if stage3:
            pending = emit_act(emit_head(0))
            fin = None
            for sb in range(1, n_super):
                nxt = emit_head(sb * SUPER)
                fin = emit_tail(pending, late_b=True)
                pending = emit_act(nxt, prev_b=(fin[3], fin[2]))
                emit_finish(fin)
            fin = emit_tail(pending, late_b=True)
            nc.scalar.activation(
                fin[2][:, 1:T],
                fin[3][:, 0 : T - 1, S - 1],
                mybir.ActivationFunctionType.Exp,
            )
            emit_finish(fin)
        else:
            pending = emit_act(emit_head(0))
            for sb in range(1, n_super):
                nxt = emit_act(emit_head(sb * SUPER))
                emit_finish(emit_tail(pending))
                pending = nxt
            emit_finish(emit_tail(pending))
    nc.compile()
    return nc


_NC_CACHE: dict = {}


def _get_nc(**kwargs):
    key = tuple(sorted(kwargs.items()))
    if key not in _NC_CACHE:
        _NC_CACHE[key] = build_nerf_bass(**kwargs)
    return _NC_CACHE[key]


def kernel(rgbo: np.ndarray, depth: np.ndarray, build_kwargs=None, **run_kwargs) -> np.ndarray:
    rgbo = np.ascontiguousarray(rgbo, dtype=np.float32)
    depth = np.ascontiguousarray(depth, dtype=np.float32)
    assert rgbo.shape == (N_RAYS, S, 4) and depth.shape == (N_RAYS, S)

    nc = _get_nc(**(build_kwargs or {}))
    # Host-side prep: channel-major rgb pre-cast to bf16 (the on-chip tanh
    # emits bf16 anyway), opacity split out in f32 for the exact cumsum.
    import ml_dtypes

    rgbo_cm = rgbo.transpose(0, 2, 1)  # [N, 4, S] view
    rgb_bf = np.ascontiguousarray(rgbo_cm[:, :3]).astype(ml_dtypes.bfloat16)
    opac = np.ascontiguousarray(rgbo_cm[:, 3])
    in_maps = []
    for i in range(N_CORES):
        sl = slice(i * NC_RAYS, (i + 1) * NC_RAYS)
        in_maps.append({"rgb": rgb_bf[sl], "opac": opac[sl], "depth": depth[sl]})
    res = run_bass_kernel_spmd(nc, in_maps, core_ids=list(range(N_CORES)), **run_kwargs)
    out = np.concatenate([r["out"] for r in res.results], axis=0)
    if run_kwargs:
        kernel.last_results = res  # stash for profiling harnesses
    return out
